# revision 29
# baseline (speedup 1.0000x reference)
"""Causal multi-head attention with RoPE on 8 TRN2 NeuronCores.

Problem (hardcoded): x [2, 2048, 1024] f32, W_qkv [1024, 3072], W_o [1024, 1024],
16 heads x 64 dh, RoPE base 10000, causal softmax attention, o-projection.

Sharding: core c = 4*b + g handles batch b (2) and head group g (4 heads).
Per core (all matmul data bf16, PSUM f32):
  - qkT projection in transposed layout ([dh, seq]): lhsT=W_qk tiles, rhs=x^T
  - v projection in natural layout ([seq, dh]), scatter-evicted into a
    [t, 128]-per-head block whose upper 64 columns are memset to 1.0 -- the
    P@V matmul then produces 64 copies of the softmax denominator in psum
    rows 64-127, partition-aligned with z (rows 0-63), so normalization is
    just reciprocal + multiply (no cross-partition broadcast needed)
  - RoPE: rotate_half as a 128x128 block-diagonal +-1 matmul on PE, then
    q' = q*cos + rot*sin on DVE (tables host-precomputed, bf16 2x mode)
  - scores^T tiles [t=128, q=512], causal-trimmed; full (non-diagonal) tiles
    computed two-at-a-time into one [128, 1024] psum so a single ACT exp
    (scale=1/8) covers both; diagonal tiles column-sliced to [c0:512] with a
    triangular mask multiply on the ragged 128-block only
  - AllToAll (8 cores, 2 rounds of 1024 q): every core ends up with all 16
    heads' z^T for 128 rows of EACH batch per round -> local o-proj with the
    full W_o, no all-reduce needed; round 0 overlaps attention chunks 2-3.
Host reassembles: core k -> rows [128k:128k+128] and [1024+128k:1024+128k+128]
of each batch.
"""

import numpy as np
import ml_dtypes

import concourse.bass as bass
import concourse.mybir as mybir
import concourse.tile as tile
from concourse.bass_utils import run_bass_kernel_spmd

BF16 = mybir.dt.bfloat16
F32 = mybir.dt.float32
AF = mybir.ActivationFunctionType

B, S, D = 2, 2048, 1024
H, DH = 16, 64
HPC = 4            # heads per core
N_CORES = 8
ROPE_BASE = 10000.0

NQ = S // 512      # 4 q-chunks of 512
SPLIT_TAIL = False
NT = S // 128      # 16 t-tiles of 128


def _npbf(a):
    return np.ascontiguousarray(a).astype(ml_dtypes.bfloat16)


def split_excess_waits(nc, limit=1):
    """walrus codegen encodes at most ~1 sync wait on most instruction structs
    (Matmult-with-embedded-ldweights, CollectiveCompute, Drain...).  Move
    excess waits onto standalone EventSemaphore instructions just before, on
    the same engine; sequencers process instructions in order so semantics
    are identical."""
    for fn in nc.m.functions:
        for bb in fn.blocks:
            out = []
            for inst in bb.instructions:
                si = inst.sync_info
                waits = list(si.on_wait) if si is not None and si.on_wait else []
                if len(waits) > limit:
                    keep = waits[len(waits) - limit:]
                    for k, w in enumerate(waits[: len(waits) - limit]):
                        ev = mybir.InstEventSemaphore(name=f"{inst.name}-wsp{k}")
                        ev.engine = inst.engine
                        ev.sync_info = mybir.SyncInfo(on_wait=[w], on_update=[])
                        out.append(ev)
                    si.on_wait = keep
                    inst.sync_info = si
                out.append(inst)
            bb.instructions = out
    return nc


def build_nc(reps=1, for_sim=False, pair_exp=True, ps_a_bufs=2, ps_z_bufs=2, ps_b_bufs=2, pt_bufs=6, dma_first=False, skip_scale=False, skip_mask=False, skip_oproj=False, skip_a2a=False, mask_pool=False, split_tail=False, v_first=False, interleave=False, tail_cols=1024):
    nc = bass.Bass()

    xt = nc.declare_dram_parameter("xt", [D, S], BF16, isOutput=False)
    wqk = nc.declare_dram_parameter("wqk", [D, 512], BF16, isOutput=False)
    wv = nc.declare_dram_parameter("wv", [D, HPC * 64], BF16, isOutput=False)
    wo = nc.declare_dram_parameter("wo", [D, D], BF16, isOutput=False)
    cosp = nc.declare_dram_parameter("cosp", [128, S], BF16, isOutput=False)
    sinp = nc.declare_dram_parameter("sinp", [128, S], BF16, isOutput=False)
    rotm = nc.declare_dram_parameter("rotm", [128, 128], BF16, isOutput=False)
    trim = nc.declare_dram_parameter("trim", [128, 128], BF16, isOutput=False)
    out = nc.declare_dram_parameter("out", [B, 256, D], F32, isOutput=True)

    with tile.TileContext(nc) as tc:
        with (
            tc.tile_pool(name="const", bufs=1) as cpool,
            tc.tile_pool(name="work", bufs=1) as wpool,
            tc.tile_pool(name="str", bufs=3) as spool,
            tc.tile_pool(name="ptp", bufs=pt_bufs) as ptpool,
            tc.tile_pool(name="ztp", bufs=2) as ztpool,
            tc.tile_pool(name="psA", bufs=ps_a_bufs, space="PSUM") as ppa,
            tc.tile_pool(name="psZ", bufs=ps_z_bufs, space="PSUM") as ppz,
            tc.tile_pool(name="psB", bufs=ps_b_bufs, space="PSUM") as ppb,
            tc.tile_pool(name="dram", bufs=1, space="DRAM") as dpool,
        ):
            # ---- constant / input loads ----
            wqk_sb = cpool.tile([128, 8, 512], BF16)
            wv_sb = cpool.tile([128, 8, HPC * 64], BF16)
            cos_sb = cpool.tile([128, S], BF16)
            sin_sb = cpool.tile([128, S], BF16)
            rot_sb = cpool.tile([128, 128], BF16)
            tri_sb = cpool.tile([128, 128], BF16)
            wo_sb = cpool.tile([128, 8, D], BF16)
            xt_sb = wpool.tile([128, 8, S], BF16)

            def load_xt_wqk():
                nc.sync.dma_start(wqk_sb[:], wqk.rearrange("(kd p) e -> p kd e", p=128))
                xt_r = xt.rearrange("(kd p) s -> kd p s", p=128)
                for kd in range(8):
                    nc.sync.dma_start(xt_sb[:, kd, 0:512], xt_r[kd][:, 0:512])
                for kd in range(8):
                    nc.sync.dma_start(xt_sb[:, kd, 512:S], xt_r[kd][:, 512:S])

            def load_rest():
                nc.sync.dma_start(wv_sb[:], wv.rearrange("(kd p) e -> p kd e", p=128))
                nc.sync.dma_start(rot_sb[:], rotm[:])
                nc.sync.dma_start(cos_sb[:], cosp[:])
                nc.sync.dma_start(sin_sb[:], sinp[:])
                nc.sync.dma_start(tri_sb[:], trim[:])
                nc.sync.dma_start(wo_sb[:], wo.rearrange("(ft p) m -> p ft m", p=128))

            if dma_first:
                load_xt_wqk()
                load_rest()
            else:
                load_xt_wqk()
                load_rest()

            # ---- phase 1: qk projection (transposed) + v projection ----
            qk_raw = wpool.tile([128, 4, S], BF16)  # m=0..3: Qh01,Qh23,Kh01,Kh23
            v_sb = wpool.tile([128, NT, HPC * 128], BF16)

            def v_tile(tt):
                ps = ppb.tile([128, HPC * 64], F32, tag="ps_b")
                for kd in range(8):
                    nc.tensor.matmul(
                        ps[:],
                        xt_sb[:, kd, tt * 128:(tt + 1) * 128],
                        wv_sb[:, kd, :],
                        start=(kd == 0),
                        stop=(kd == 7),
                    )
                nc.vector.tensor_copy(
                    v_sb[:, tt, :].rearrange("p (h e) -> p h e", h=HPC)[:, :, 0:64],
                    ps[:].rearrange("p (h e) -> p h e", h=HPC),
                )
                # 64 ones columns per head: psz rows 64-127 = softmax denominator
                nc.vector.memset(
                    v_sb[:, tt, :].rearrange("p (h e) -> p h e", h=HPC)[:, :, 64:128],
                    1.0,
                )

            def qk_tile(n, m):
                ps = ppa.tile([128, 512], F32, tag="ps_a")
                for kd in range(8):
                    nc.tensor.matmul(
                        ps[:],
                        wqk_sb[:, kd, m * 128:(m + 1) * 128],
                        xt_sb[:, kd, n * 512:(n + 1) * 512],
                        start=(kd == 0),
                        stop=(kd == 7),
                    )
                nc.vector.tensor_copy(qk_raw[:, m, n * 512:(n + 1) * 512], ps[:])

            if v_first:
                for tt in range(4):
                    v_tile(tt)
                for n in range(NQ):
                    for m in range(4):
                        qk_tile(n, m)
                for tt in range(4, NT):
                    v_tile(tt)
            else:
                for n in range(NQ):
                    for m in range(4):
                        qk_tile(n, m)
                for tt in range(NT):
                    v_tile(tt)

            # ---- phase 2: RoPE on q,k ----
            qkr = wpool.tile([128, 4, S], BF16)
            for n in range(NQ):
                for m in range(4):
                    sl = slice(n * 512, (n + 1) * 512)
                    ps = ppa.tile([128, 512], F32, tag="ps_a")
                    nc.tensor.matmul(
                        ps[:], rot_sb[:], qk_raw[:, m, sl], start=True, stop=True
                    )
                    rsb = spool.tile([128, 512], BF16, tag="ropebuf")
                    nc.scalar.activation(rsb[:], ps[:], AF.Copy)
                    t1 = spool.tile([128, 512], BF16, tag="ropet1")
                    nc.vector.tensor_mul(t1[:], rsb[:], sin_sb[:, sl])
                    t2 = spool.tile([128, 512], BF16, tag="ropet2")
                    nc.vector.tensor_mul(t2[:], qk_raw[:, m, sl], cos_sb[:, sl])
                    nc.vector.tensor_add(qkr[:, m, sl], t1[:], t2[:])


            zt_holder = {}

            def attention_chunk(i):
                zt_sb = zt_holder["zt"]
                for h in range(HPC):
                    rows = slice(64 * (h % 2), 64 * (h % 2) + 64)
                    qm, km = h // 2, 2 + h // 2
                    qsl = slice(i * 512, (i + 1) * 512)
                    psz = ppz.tile([128, 512], F32, tag="ps_z")
                    last_j = 4 * i + 3
                    started = False
                    if not pair_exp:
                        for j in range(4 * i):
                            pss = ppa.tile([128, 512], F32, tag="ps_a")
                            nc.tensor.matmul(
                                pss[:],
                                qkr[rows, km, j * 128:(j + 1) * 128],
                                qkr[rows, qm, qsl],
                                start=True, stop=True,
                            )
                            pt = ptpool.tile([128, 512], BF16, tag="pt")
                            nc.scalar.activation(pt[:], pss[:], AF.Exp, scale=0.125)
                            nc.tensor.matmul(
                                psz[:],
                                v_sb[:, j, 128 * h:128 * h + 128],
                                pt[:],
                                start=not started, stop=False,
                            )
                            started = True
                    # full tiles in pairs: one [128, 1024] psum, one exp op
                    for ja in [] if not pair_exp else range(0, 4 * i, 2):
                        jb = ja + 1
                        ps2 = ppa.tile([128, 1024], F32, tag="ps_a")
                        nc.tensor.matmul(
                            ps2[:, 0:512],
                            qkr[rows, km, ja * 128:(ja + 1) * 128],
                            qkr[rows, qm, qsl],
                            start=True, stop=True,
                        )
                        nc.tensor.matmul(
                            ps2[:, 512:1024],
                            qkr[rows, km, jb * 128:(jb + 1) * 128],
                            qkr[rows, qm, qsl],
                            start=True, stop=True,
                        )
                        pt2 = ptpool.tile([128, 1024], BF16, tag="pt")
                        nc.scalar.activation(pt2[:], ps2[:], AF.Exp, scale=0.125)
                        nc.tensor.matmul(
                            psz[:],
                            v_sb[:, ja, 128 * h:128 * h + 128],
                            pt2[:, 0:512],
                            start=not started, stop=False,
                        )
                        started = True
                        nc.tensor.matmul(
                            psz[:],
                            v_sb[:, jb, 128 * h:128 * h + 128],
                            pt2[:, 512:1024],
                            start=False, stop=False,
                        )
                    # diagonal tiles: ragged, column-sliced
                    for j in range(4 * i, 4 * i + 4):
                        r = j - 4 * i
                        c0 = 128 * r
                        pss = ppa.tile([128, 512], F32, tag="ps_a")
                        nc.tensor.matmul(
                            pss[:, c0:512],
                            qkr[rows, km, j * 128:(j + 1) * 128],
                            qkr[rows, qm, qsl][:, c0:512],
                            start=True, stop=True,
                        )
                        pt = ptpool.tile([128, 512], BF16, tag="pt")
                        nc.scalar.activation(
                            pt[:, c0:512], pss[:, c0:512], AF.Exp, scale=0.125
                        )
                        if not skip_mask:
                            eng = nc.gpsimd if mask_pool else nc.vector
                            eng.tensor_tensor(
                                pt[:, c0:c0 + 128], pt[:, c0:c0 + 128], tri_sb[:],
                                mybir.AluOpType.mult,
                            )
                        nc.tensor.matmul(
                            psz[:, c0:512],
                            v_sb[:, j, 128 * h:128 * h + 128],
                            pt[:, c0:512],
                            start=(not started and c0 == 0), stop=(j == last_j),
                        )
                        started = True
                    # psz rows 64-127 are 64 copies of the denominator row
                    if skip_scale:
                        nc.vector.tensor_copy(
                            zt_sb[rows, h // 2, i * 512:(i + 1) * 512], psz[0:64, :])
                        continue
                    rsh = spool.tile([64, 512], F32, tag="rsh")
                    nc.vector.reciprocal(rsh[:], psz[64:128, :])
                    nc.vector.tensor_mul(
                        zt_sb[rows, h // 2, i * 512:(i + 1) * 512],
                        psz[0:64, :],
                        rsh[:],
                    )

            def attention_chunk_il(i):
                zt_sb = zt_holder["zt"]
                qsl = slice(i * 512, (i + 1) * 512)
                last_j = 4 * i + 3
                for hp in range(2):
                    hA, hB = 2 * hp, 2 * hp + 1
                    rowsA = slice(0, 64)
                    rowsB = slice(64, 128)
                    qm, km = hp, 2 + hp
                    pszA = ppz.tile([128, 512], F32, tag="ps_z")
                    pszB = ppz.tile([128, 512], F32, tag="ps_z")
                    startedA = False
                    for ja in range(0, 4 * i, 2):
                        jb = ja + 1
                        for rows, h, psz in ((rowsA, hA, pszA), (rowsB, hB, pszB)):
                            ps2 = ppa.tile([128, 1024], F32, tag="ps_a")
                            nc.tensor.matmul(
                                ps2[:, 0:512],
                                qkr[rows, km, ja * 128:(ja + 1) * 128],
                                qkr[rows, qm, qsl],
                                start=True, stop=True,
                            )
                            nc.tensor.matmul(
                                ps2[:, 512:1024],
                                qkr[rows, km, jb * 128:(jb + 1) * 128],
                                qkr[rows, qm, qsl],
                                start=True, stop=True,
                            )
                            pt2 = ptpool.tile([128, 1024], BF16, tag="pt")
                            nc.scalar.activation(pt2[:], ps2[:], AF.Exp, scale=0.125)
                            nc.tensor.matmul(
                                psz[:],
                                v_sb[:, ja, 128 * h:128 * h + 128],
                                pt2[:, 0:512],
                                start=not startedA, stop=False,
                            )
                            nc.tensor.matmul(
                                psz[:],
                                v_sb[:, jb, 128 * h:128 * h + 128],
                                pt2[:, 512:1024],
                                start=False, stop=False,
                            )
                        startedA = True
                    for j in range(4 * i, 4 * i + 4):
                        r = j - 4 * i
                        c0 = 128 * r
                        for rows, h, psz in ((rowsA, hA, pszA), (rowsB, hB, pszB)):
                            pss = ppa.tile([128, 512], F32, tag="ps_a")
                            nc.tensor.matmul(
                                pss[:, c0:512],
                                qkr[rows, km, j * 128:(j + 1) * 128],
                                qkr[rows, qm, qsl][:, c0:512],
                                start=True, stop=True,
                            )
                            pt = ptpool.tile([128, 512], BF16, tag="pt")
                            nc.scalar.activation(
                                pt[:, c0:512], pss[:, c0:512], AF.Exp, scale=0.125
                            )
                            nc.vector.tensor_mul(
                                pt[:, c0:c0 + 128], pt[:, c0:c0 + 128], tri_sb[:]
                            )
                            nc.tensor.matmul(
                                psz[:, c0:512],
                                v_sb[:, j, 128 * h:128 * h + 128],
                                pt[:, c0:512],
                                start=(not startedA and c0 == 0),
                                stop=(j == last_j),
                            )
                        startedA = True
                    for rows, h, psz in ((rowsA, hA, pszA), (rowsB, hB, pszB)):
                        rsh = spool.tile([64, 512], F32, tag="rsh")
                        nc.vector.reciprocal(rsh[:], psz[64:128, :])
                        nc.vector.tensor_mul(
                            zt_sb[rows, hp, i * 512:(i + 1) * 512],
                            psz[0:64, :],
                            rsh[:],
                        )

            def a2a_round(tag, col0, rw, orow):
                # exchange zt cols [col0 : col0+8*rw]; core k owns rows
                # [col0 + rw*k : +rw] of EACH batch -> out rows [orow : orow+rw].
                # o-proj packs both batches' rows along M so psum tiles stay at
                # 128 rows even when rw < 128.
                if skip_a2a:
                    return
                zt_sb = zt_holder["zt"]
                cc_in = dpool.tile([8, 256, rw], BF16, tag=f"ccin{tag}_{_rep % 2}")
                cc_out = dpool.tile([8, 256, rw], BF16, tag=f"ccout{tag}_{_rep % 2}")
                for k in range(8):
                    nc.sync.dma_start(
                        cc_in[k].rearrange("(t p) q -> p t q", p=128),
                        zt_sb[:, :, col0 + rw * k: col0 + rw * (k + 1)],
                    )
                if for_sim:
                    nc.gpsimd.dma_start(cc_out[:], cc_in[:])
                else:
                    nc.gpsimd.collective_compute(
                        "AllToAll",
                        mybir.AluOpType.bypass,
                        ins=[cc_in[:].opt()],
                        outs=[cc_out[:].opt()],
                        replica_groups=[list(range(8))],
                    )
                if skip_oproj:
                    return
                R = 2 * rw  # packed rows: [b0 rows | b1 rows]
                zf = spool.tile([128, 8, R], BF16, tag=f"zfall_{rw}")
                for i in range(8):
                    b, j = divmod(i, 4)
                    nc.sync.dma_start(
                        zf[:, 2 * j:2 * j + 2, b * rw:(b + 1) * rw],
                        cc_out[i].rearrange("(t p) q -> p t q", p=128),
                    )
                for st in range(0, R, 128):
                    rs = min(128, R - st)
                    for mc in range(2):
                        pso = ppb.tile([rs, 512], F32, tag="ps_b")
                        for ft in range(8):
                            nc.tensor.matmul(
                                pso[:],
                                zf[:, ft, st:st + rs],
                                wo_sb[:, ft, mc * 512:(mc + 1) * 512],
                                start=(ft == 0),
                                stop=(ft == 7),
                            )
                        osb = spool.tile([rs, 512], F32, tag="osb")
                        nc.any.tensor_copy(osb[:], pso[:])
                        # split the packed rows into per-batch output segments
                        g0 = st
                        while g0 < st + rs:
                            b = g0 // rw
                            g1 = min((b + 1) * rw, st + rs)
                            nc.sync.dma_start(
                                out[b, orow + (g0 - b * rw):orow + (g1 - b * rw),
                                    mc * 512:(mc + 1) * 512],
                                osb[g0 - st:g1 - st, :],
                            )
                            g0 = g1

            chunk_fn = attention_chunk_il if interleave else attention_chunk
            for _rep in range(reps):
                zt_holder["zt"] = ztpool.tile([128, 2, S], BF16, name="zt", tag="zt")
                chunk_fn(0)
                chunk_fn(1)
                if tail_cols == 512:
                    chunk_fn(2)
                    a2a_round("r0", 0, 192, 0)
                    chunk_fn(3)
                    a2a_round("r1", 1536, 64, 192)
                else:
                    a2a_round("r0", 0, 128, 0)
                    chunk_fn(2)
                    chunk_fn(3)
                    a2a_round("r1", 1024, 128, 128)

    split_excess_waits(nc)
    return nc


def prepare_in_maps(x, W_qkv, W_o):
    x = np.asarray(x, dtype=np.float32)
    W_qkv = np.asarray(W_qkv, dtype=np.float32)
    W_o = np.asarray(W_o, dtype=np.float32)

    # RoPE tables in the [2-head x dh, seq] transposed layout
    inv_freq = 1.0 / (ROPE_BASE ** (np.arange(0, DH, 2, dtype=np.float32) / DH))
    t = np.arange(S, dtype=np.float32)
    freqs = np.outer(t, inv_freq)                      # [S, 32]
    emb = np.concatenate([freqs, freqs], -1)           # [S, 64]
    cos64 = np.cos(emb).T                              # [64, S]
    sin64 = np.sin(emb).T
    cosp = np.concatenate([cos64, cos64], 0)           # [128, S]
    sinp = np.concatenate([sin64, sin64], 0)

    # rotate-half as a stationary matrix: psum_rot = rotm.T @ qT per 64-block
    r0 = np.zeros((64, 64), dtype=np.float32)
    for dd in range(32):
        r0[dd + 32, dd] = -1.0     # out[d<32] = -q[d+32]
        r0[dd, dd + 32] = 1.0      # out[d>=32] = q[d-32]
    rotm = np.zeros((128, 128), dtype=np.float32)
    rotm[:64, :64] = r0
    rotm[64:, 64:] = r0

    tt_, qq_ = np.meshgrid(np.arange(128), np.arange(128), indexing="ij")
    trim = (qq_ >= tt_).astype(np.float32)

    in_maps = []
    for c in range(N_CORES):
        b, g = c // 4, c % 4
        heads = [4 * g + hh for hh in range(HPC)]
        wqk = np.concatenate(
            [W_qkv[:, 64 * h:64 * h + 64] for h in heads]
            + [W_qkv[:, D + 64 * h:D + 64 * h + 64] for h in heads],
            axis=1,
        )
        wv = np.concatenate(
            [W_qkv[:, 2 * D + 64 * h:2 * D + 64 * h + 64] for h in heads], axis=1
        )
        in_maps.append(
            {
                "xt": _npbf(x[b].T),
                "wqk": _npbf(wqk),
                "wv": _npbf(wv),
                "wo": _npbf(W_o),
                "cosp": _npbf(cosp),
                "sinp": _npbf(sinp),
                "rotm": _npbf(rotm),
                "trim": _npbf(trim),
            }
        )
    return in_maps


TAIL_COLS = 1024


def assemble_output(results):
    full = np.empty((B, S, D), dtype=np.float32)
    for k in range(N_CORES):
        o = np.asarray(results[k]["out"], dtype=np.float32)  # [B, 256, D]
        for b in range(B):
            if TAIL_COLS == 512:
                full[b, 192 * k:192 * (k + 1)] = o[b, 0:192]
                full[b, 1536 + 64 * k:1536 + 64 * (k + 1)] = o[b, 192:256]
            else:
                full[b, 128 * k:128 * (k + 1)] = o[b, 0:128]
                full[b, 1024 + 128 * k:1024 + 128 * (k + 1)] = o[b, 128:256]
    return full


_NC_CACHE = {}


def kernel(x, W_qkv, W_o):
    key = "nc"
    if key not in _NC_CACHE:
        _NC_CACHE[key] = build_nc(tail_cols=TAIL_COLS)
    nc = _NC_CACHE[key]
    in_maps = prepare_in_maps(x, W_qkv, W_o)
    res = run_bass_kernel_spmd(nc, in_maps, core_ids=list(range(N_CORES)))
    return assemble_output(res.results)


# revision 35
# speedup vs baseline: 1.0434x; 1.0434x over previous
"""Causal multi-head attention with RoPE on 8 TRN2 NeuronCores.

Problem (hardcoded): x [2, 2048, 1024] f32, W_qkv [1024, 3072], W_o [1024, 1024],
16 heads x 64 dh, RoPE base 10000, causal softmax attention, o-projection.

Sharding: core c = 4*b + g handles batch b (2) and head group g (4 heads).
Per core (all matmul data bf16, PSUM f32):
  - qkT projection in transposed layout ([dh, seq]): lhsT=W_qk tiles, rhs=x^T
  - v projection in natural layout ([seq, dh]), scatter-evicted into a
    [t, 128]-per-head block whose upper 64 columns are memset to 1.0 -- the
    P@V matmul then produces 64 copies of the softmax denominator in psum
    rows 64-127, partition-aligned with z (rows 0-63), so normalization is
    just reciprocal + multiply (no cross-partition broadcast needed)
  - RoPE: rotate_half as a 128x128 block-diagonal +-1 matmul on PE, then
    q' = q*cos + rot*sin on DVE (tables host-precomputed, bf16 2x mode)
  - scores^T tiles [t=128, q=512], causal-trimmed; full (non-diagonal) tiles
    computed two-at-a-time into one [128, 1024] psum so a single ACT exp
    (scale=1/8) covers both; diagonal tiles column-sliced to [c0:512] with a
    triangular mask multiply on the ragged 128-block only
  - AllToAll (8 cores, 2 rounds of 1024 q): every core ends up with all 16
    heads' z^T for 128 rows of EACH batch per round -> local o-proj with the
    full W_o, no all-reduce needed; round 0 overlaps attention chunks 2-3.
Host reassembles: core k -> rows [128k:128k+128] and [1024+128k:1024+128k+128]
of each batch.
"""

import numpy as np
import ml_dtypes

import concourse.bass as bass
import concourse.mybir as mybir
import concourse.tile as tile
from concourse.bass_utils import run_bass_kernel_spmd

BF16 = mybir.dt.bfloat16
F32 = mybir.dt.float32
AF = mybir.ActivationFunctionType

B, S, D = 2, 2048, 1024
H, DH = 16, 64
HPC = 4            # heads per core
N_CORES = 8
ROPE_BASE = 10000.0

NQ = S // 512      # 4 q-chunks of 512
SPLIT_TAIL = False
NT = S // 128      # 16 t-tiles of 128


def _npbf(a):
    return np.ascontiguousarray(a).astype(ml_dtypes.bfloat16)


def split_excess_waits(nc, limit=1):
    """walrus codegen encodes at most ~1 sync wait on most instruction structs
    (Matmult-with-embedded-ldweights, CollectiveCompute, Drain...).  Move
    excess waits onto standalone EventSemaphore instructions just before, on
    the same engine; sequencers process instructions in order so semantics
    are identical."""
    for fn in nc.m.functions:
        for bb in fn.blocks:
            out = []
            for inst in bb.instructions:
                si = inst.sync_info
                waits = list(si.on_wait) if si is not None and si.on_wait else []
                if len(waits) > limit:
                    keep = waits[len(waits) - limit:]
                    for k, w in enumerate(waits[: len(waits) - limit]):
                        ev = mybir.InstEventSemaphore(name=f"{inst.name}-wsp{k}")
                        ev.engine = inst.engine
                        ev.sync_info = mybir.SyncInfo(on_wait=[w], on_update=[])
                        out.append(ev)
                    si.on_wait = keep
                    inst.sync_info = si
                out.append(inst)
            bb.instructions = out
    return nc


def build_nc(reps=1, for_sim=False, pair_exp=False, ps_a_bufs=4, ps_z_bufs=3, ps_b_bufs=1, pt_bufs=6, dma_first=False, skip_scale=False, skip_mask=False, skip_oproj=False, skip_a2a=False, mask_pool=False, split_tail=False, v_first=False, interleave=False, tail_cols=1024):
    nc = bass.Bass()

    xt = nc.declare_dram_parameter("xt", [D, S], BF16, isOutput=False)
    wqk = nc.declare_dram_parameter("wqk", [D, 512], BF16, isOutput=False)
    wv = nc.declare_dram_parameter("wv", [D, HPC * 64], BF16, isOutput=False)
    wo = nc.declare_dram_parameter("wo", [D, D], BF16, isOutput=False)
    cosp = nc.declare_dram_parameter("cosp", [128, S], BF16, isOutput=False)
    sinp = nc.declare_dram_parameter("sinp", [128, S], BF16, isOutput=False)
    rotm = nc.declare_dram_parameter("rotm", [128, 128], BF16, isOutput=False)
    trim = nc.declare_dram_parameter("trim", [128, 128], BF16, isOutput=False)
    out = nc.declare_dram_parameter("out", [B, 256, D], F32, isOutput=True)

    with tile.TileContext(nc) as tc:
        with (
            tc.tile_pool(name="const", bufs=1) as cpool,
            tc.tile_pool(name="work", bufs=1) as wpool,
            tc.tile_pool(name="str", bufs=3) as spool,
            tc.tile_pool(name="ptp", bufs=pt_bufs) as ptpool,
            tc.tile_pool(name="ztp", bufs=2) as ztpool,
            tc.tile_pool(name="psA", bufs=ps_a_bufs, space="PSUM") as ppa,
            tc.tile_pool(name="psZ", bufs=ps_z_bufs, space="PSUM") as ppz,
            tc.tile_pool(name="psB", bufs=ps_b_bufs, space="PSUM") as ppb,
            tc.tile_pool(name="dram", bufs=1, space="DRAM") as dpool,
        ):
            # ---- constant / input loads ----
            wqk_sb = cpool.tile([128, 8, 512], BF16)
            wv_sb = cpool.tile([128, 8, HPC * 64], BF16)
            cos_sb = cpool.tile([128, S], BF16)
            sin_sb = cpool.tile([128, S], BF16)
            rot_sb = cpool.tile([128, 128], BF16)
            tri_sb = cpool.tile([128, 128], BF16)
            wo_sb = cpool.tile([128, 8, D], BF16)
            xt_sb = wpool.tile([128, 8, S], BF16)

            def load_xt_wqk():
                nc.sync.dma_start(wqk_sb[:], wqk.rearrange("(kd p) e -> p kd e", p=128))
                xt_r = xt.rearrange("(kd p) s -> kd p s", p=128)
                for kd in range(8):
                    nc.sync.dma_start(xt_sb[:, kd, 0:512], xt_r[kd][:, 0:512])
                for kd in range(8):
                    nc.sync.dma_start(xt_sb[:, kd, 512:S], xt_r[kd][:, 512:S])

            def load_rest():
                nc.sync.dma_start(wv_sb[:], wv.rearrange("(kd p) e -> p kd e", p=128))
                nc.sync.dma_start(rot_sb[:], rotm[:])
                nc.sync.dma_start(cos_sb[:], cosp[:])
                nc.sync.dma_start(sin_sb[:], sinp[:])
                nc.sync.dma_start(tri_sb[:], trim[:])
                nc.sync.dma_start(wo_sb[:], wo.rearrange("(ft p) m -> p ft m", p=128))

            if dma_first:
                load_xt_wqk()
                load_rest()
            else:
                load_xt_wqk()
                load_rest()

            # ---- phase 1: qk projection (transposed) + v projection ----
            qk_raw = wpool.tile([128, 4, S], BF16)  # m=0..3: Qh01,Qh23,Kh01,Kh23
            v_sb = wpool.tile([128, NT, HPC * 128], BF16)

            def v_tile(tt):
                ps = ppb.tile([128, HPC * 64], F32, tag="ps_b")
                for kd in range(8):
                    nc.tensor.matmul(
                        ps[:],
                        xt_sb[:, kd, tt * 128:(tt + 1) * 128],
                        wv_sb[:, kd, :],
                        start=(kd == 0),
                        stop=(kd == 7),
                    )
                nc.vector.tensor_copy(
                    v_sb[:, tt, :].rearrange("p (h e) -> p h e", h=HPC)[:, :, 0:64],
                    ps[:].rearrange("p (h e) -> p h e", h=HPC),
                )
                # 64 ones columns per head: psz rows 64-127 = softmax denominator
                nc.vector.memset(
                    v_sb[:, tt, :].rearrange("p (h e) -> p h e", h=HPC)[:, :, 64:128],
                    1.0,
                )

            def qk_tile(n, m):
                ps = ppa.tile([128, 512], F32, tag="ps_a")
                for kd in range(8):
                    nc.tensor.matmul(
                        ps[:],
                        wqk_sb[:, kd, m * 128:(m + 1) * 128],
                        xt_sb[:, kd, n * 512:(n + 1) * 512],
                        start=(kd == 0),
                        stop=(kd == 7),
                    )
                nc.vector.tensor_copy(qk_raw[:, m, n * 512:(n + 1) * 512], ps[:])

            if v_first:
                for tt in range(4):
                    v_tile(tt)
                for n in range(NQ):
                    for m in range(4):
                        qk_tile(n, m)
                for tt in range(4, NT):
                    v_tile(tt)
            else:
                for n in range(NQ):
                    for m in range(4):
                        qk_tile(n, m)
                for tt in range(NT):
                    v_tile(tt)

            # ---- phase 2: RoPE on q,k ----
            qkr = wpool.tile([128, 4, S], BF16)
            for n in range(NQ):
                for m in range(4):
                    sl = slice(n * 512, (n + 1) * 512)
                    ps = ppa.tile([128, 512], F32, tag="ps_a")
                    nc.tensor.matmul(
                        ps[:], rot_sb[:], qk_raw[:, m, sl], start=True, stop=True
                    )
                    rsb = spool.tile([128, 512], BF16, tag="ropebuf")
                    nc.scalar.activation(rsb[:], ps[:], AF.Copy)
                    t1 = spool.tile([128, 512], BF16, tag="ropet1")
                    nc.vector.tensor_mul(t1[:], rsb[:], sin_sb[:, sl])
                    t2 = spool.tile([128, 512], BF16, tag="ropet2")
                    nc.vector.tensor_mul(t2[:], qk_raw[:, m, sl], cos_sb[:, sl])
                    nc.vector.tensor_add(qkr[:, m, sl], t1[:], t2[:])


            zt_holder = {}

            def attention_chunk(i):
                zt_sb = zt_holder["zt"]
                for h in range(HPC):
                    rows = slice(64 * (h % 2), 64 * (h % 2) + 64)
                    qm, km = h // 2, 2 + h // 2
                    qsl = slice(i * 512, (i + 1) * 512)
                    psz = ppz.tile([128, 512], F32, tag="ps_z")
                    last_j = 4 * i + 3
                    started = False
                    if not pair_exp:
                        for j in range(4 * i):
                            pss = ppa.tile([128, 512], F32, tag="ps_a")
                            nc.tensor.matmul(
                                pss[:],
                                qkr[rows, km, j * 128:(j + 1) * 128],
                                qkr[rows, qm, qsl],
                                start=True, stop=True,
                            )
                            pt = ptpool.tile([128, 512], BF16, tag="pt")
                            nc.scalar.activation(pt[:], pss[:], AF.Exp, scale=0.125)
                            nc.tensor.matmul(
                                psz[:],
                                v_sb[:, j, 128 * h:128 * h + 128],
                                pt[:],
                                start=not started, stop=False,
                            )
                            started = True
                    # full tiles in pairs: one [128, 1024] psum, one exp op
                    for ja in [] if not pair_exp else range(0, 4 * i, 2):
                        jb = ja + 1
                        ps2 = ppa.tile([128, 1024], F32, tag="ps_a")
                        nc.tensor.matmul(
                            ps2[:, 0:512],
                            qkr[rows, km, ja * 128:(ja + 1) * 128],
                            qkr[rows, qm, qsl],
                            start=True, stop=True,
                        )
                        nc.tensor.matmul(
                            ps2[:, 512:1024],
                            qkr[rows, km, jb * 128:(jb + 1) * 128],
                            qkr[rows, qm, qsl],
                            start=True, stop=True,
                        )
                        pt2 = ptpool.tile([128, 1024], BF16, tag="pt")
                        nc.scalar.activation(pt2[:], ps2[:], AF.Exp, scale=0.125)
                        nc.tensor.matmul(
                            psz[:],
                            v_sb[:, ja, 128 * h:128 * h + 128],
                            pt2[:, 0:512],
                            start=not started, stop=False,
                        )
                        started = True
                        nc.tensor.matmul(
                            psz[:],
                            v_sb[:, jb, 128 * h:128 * h + 128],
                            pt2[:, 512:1024],
                            start=False, stop=False,
                        )
                    # diagonal tiles: ragged, column-sliced
                    for j in range(4 * i, 4 * i + 4):
                        r = j - 4 * i
                        c0 = 128 * r
                        pss = ppa.tile([128, 512], F32, tag="ps_a")
                        nc.tensor.matmul(
                            pss[:, c0:512],
                            qkr[rows, km, j * 128:(j + 1) * 128],
                            qkr[rows, qm, qsl][:, c0:512],
                            start=True, stop=True,
                        )
                        pt = ptpool.tile([128, 512], BF16, tag="pt")
                        nc.scalar.activation(
                            pt[:, c0:512], pss[:, c0:512], AF.Exp, scale=0.125
                        )
                        if not skip_mask:
                            eng = nc.gpsimd if mask_pool else nc.vector
                            eng.tensor_tensor(
                                pt[:, c0:c0 + 128], pt[:, c0:c0 + 128], tri_sb[:],
                                mybir.AluOpType.mult,
                            )
                        nc.tensor.matmul(
                            psz[:, c0:512],
                            v_sb[:, j, 128 * h:128 * h + 128],
                            pt[:, c0:512],
                            start=(not started and c0 == 0), stop=(j == last_j),
                        )
                        started = True
                    # psz rows 64-127 are 64 copies of the denominator row
                    if skip_scale:
                        nc.vector.tensor_copy(
                            zt_sb[rows, h // 2, i * 512:(i + 1) * 512], psz[0:64, :])
                        continue
                    rsh = spool.tile([64, 512], F32, tag="rsh")
                    nc.vector.reciprocal(rsh[:], psz[64:128, :])
                    nc.vector.tensor_mul(
                        zt_sb[rows, h // 2, i * 512:(i + 1) * 512],
                        psz[0:64, :],
                        rsh[:],
                    )

            def attention_chunk_il(i):
                zt_sb = zt_holder["zt"]
                qsl = slice(i * 512, (i + 1) * 512)
                last_j = 4 * i + 3
                for hp in range(2):
                    hA, hB = 2 * hp, 2 * hp + 1
                    rowsA = slice(0, 64)
                    rowsB = slice(64, 128)
                    qm, km = hp, 2 + hp
                    pszA = ppz.tile([128, 512], F32, tag="ps_z")
                    pszB = ppz.tile([128, 512], F32, tag="ps_z")
                    startedA = False
                    for ja in range(0, 4 * i, 2):
                        jb = ja + 1
                        for rows, h, psz in ((rowsA, hA, pszA), (rowsB, hB, pszB)):
                            ps2 = ppa.tile([128, 1024], F32, tag="ps_a")
                            nc.tensor.matmul(
                                ps2[:, 0:512],
                                qkr[rows, km, ja * 128:(ja + 1) * 128],
                                qkr[rows, qm, qsl],
                                start=True, stop=True,
                            )
                            nc.tensor.matmul(
                                ps2[:, 512:1024],
                                qkr[rows, km, jb * 128:(jb + 1) * 128],
                                qkr[rows, qm, qsl],
                                start=True, stop=True,
                            )
                            pt2 = ptpool.tile([128, 1024], BF16, tag="pt")
                            nc.scalar.activation(pt2[:], ps2[:], AF.Exp, scale=0.125)
                            nc.tensor.matmul(
                                psz[:],
                                v_sb[:, ja, 128 * h:128 * h + 128],
                                pt2[:, 0:512],
                                start=not startedA, stop=False,
                            )
                            nc.tensor.matmul(
                                psz[:],
                                v_sb[:, jb, 128 * h:128 * h + 128],
                                pt2[:, 512:1024],
                                start=False, stop=False,
                            )
                        startedA = True
                    for j in range(4 * i, 4 * i + 4):
                        r = j - 4 * i
                        c0 = 128 * r
                        for rows, h, psz in ((rowsA, hA, pszA), (rowsB, hB, pszB)):
                            pss = ppa.tile([128, 512], F32, tag="ps_a")
                            nc.tensor.matmul(
                                pss[:, c0:512],
                                qkr[rows, km, j * 128:(j + 1) * 128],
                                qkr[rows, qm, qsl][:, c0:512],
                                start=True, stop=True,
                            )
                            pt = ptpool.tile([128, 512], BF16, tag="pt")
                            nc.scalar.activation(
                                pt[:, c0:512], pss[:, c0:512], AF.Exp, scale=0.125
                            )
                            nc.vector.tensor_mul(
                                pt[:, c0:c0 + 128], pt[:, c0:c0 + 128], tri_sb[:]
                            )
                            nc.tensor.matmul(
                                psz[:, c0:512],
                                v_sb[:, j, 128 * h:128 * h + 128],
                                pt[:, c0:512],
                                start=(not startedA and c0 == 0),
                                stop=(j == last_j),
                            )
                        startedA = True
                    for rows, h, psz in ((rowsA, hA, pszA), (rowsB, hB, pszB)):
                        rsh = spool.tile([64, 512], F32, tag="rsh")
                        nc.vector.reciprocal(rsh[:], psz[64:128, :])
                        nc.vector.tensor_mul(
                            zt_sb[rows, hp, i * 512:(i + 1) * 512],
                            psz[0:64, :],
                            rsh[:],
                        )

            def a2a_round(tag, col0, rw, orow):
                # exchange zt cols [col0 : col0+8*rw]; core k owns rows
                # [col0 + rw*k : +rw] of EACH batch -> out rows [orow : orow+rw].
                # o-proj packs both batches' rows along M so psum tiles stay at
                # 128 rows even when rw < 128.
                if skip_a2a:
                    return
                zt_sb = zt_holder["zt"]
                cc_in = dpool.tile([8, 256, rw], BF16, tag=f"ccin{tag}_{_rep % 2}")
                cc_out = dpool.tile([8, 256, rw], BF16, tag=f"ccout{tag}_{_rep % 2}")
                for k in range(8):
                    nc.sync.dma_start(
                        cc_in[k].rearrange("(t p) q -> p t q", p=128),
                        zt_sb[:, :, col0 + rw * k: col0 + rw * (k + 1)],
                    )
                if for_sim:
                    nc.gpsimd.dma_start(cc_out[:], cc_in[:])
                else:
                    nc.gpsimd.collective_compute(
                        "AllToAll",
                        mybir.AluOpType.bypass,
                        ins=[cc_in[:].opt()],
                        outs=[cc_out[:].opt()],
                        replica_groups=[list(range(8))],
                    )
                if skip_oproj:
                    return
                R = 2 * rw  # packed rows: [b0 rows | b1 rows]
                zf = spool.tile([128, 8, R], BF16, tag=f"zfall_{rw}")
                for i in range(8):
                    b, j = divmod(i, 4)
                    nc.sync.dma_start(
                        zf[:, 2 * j:2 * j + 2, b * rw:(b + 1) * rw],
                        cc_out[i].rearrange("(t p) q -> p t q", p=128),
                    )
                for st in range(0, R, 128):
                    rs = min(128, R - st)
                    for mc in range(2):
                        pso = ppb.tile([rs, 512], F32, tag="ps_b")
                        for ft in range(8):
                            nc.tensor.matmul(
                                pso[:],
                                zf[:, ft, st:st + rs],
                                wo_sb[:, ft, mc * 512:(mc + 1) * 512],
                                start=(ft == 0),
                                stop=(ft == 7),
                            )
                        osb = spool.tile([rs, 512], F32, tag="osb")
                        nc.any.tensor_copy(osb[:], pso[:])
                        # split the packed rows into per-batch output segments
                        g0 = st
                        while g0 < st + rs:
                            b = g0 // rw
                            g1 = min((b + 1) * rw, st + rs)
                            nc.sync.dma_start(
                                out[b, orow + (g0 - b * rw):orow + (g1 - b * rw),
                                    mc * 512:(mc + 1) * 512],
                                osb[g0 - st:g1 - st, :],
                            )
                            g0 = g1

            chunk_fn = attention_chunk_il if interleave else attention_chunk
            for _rep in range(reps):
                zt_holder["zt"] = ztpool.tile([128, 2, S], BF16, name="zt", tag="zt")
                chunk_fn(0)
                chunk_fn(1)
                if tail_cols == 512:
                    chunk_fn(2)
                    a2a_round("r0", 0, 192, 0)
                    chunk_fn(3)
                    a2a_round("r1", 1536, 64, 192)
                else:
                    a2a_round("r0", 0, 128, 0)
                    chunk_fn(2)
                    chunk_fn(3)
                    a2a_round("r1", 1024, 128, 128)

    split_excess_waits(nc)
    return nc


def prepare_in_maps(x, W_qkv, W_o):
    x = np.asarray(x, dtype=np.float32)
    W_qkv = np.asarray(W_qkv, dtype=np.float32)
    W_o = np.asarray(W_o, dtype=np.float32)

    # RoPE tables in the [2-head x dh, seq] transposed layout
    inv_freq = 1.0 / (ROPE_BASE ** (np.arange(0, DH, 2, dtype=np.float32) / DH))
    t = np.arange(S, dtype=np.float32)
    freqs = np.outer(t, inv_freq)                      # [S, 32]
    emb = np.concatenate([freqs, freqs], -1)           # [S, 64]
    cos64 = np.cos(emb).T                              # [64, S]
    sin64 = np.sin(emb).T
    cosp = np.concatenate([cos64, cos64], 0)           # [128, S]
    sinp = np.concatenate([sin64, sin64], 0)

    # rotate-half as a stationary matrix: psum_rot = rotm.T @ qT per 64-block
    r0 = np.zeros((64, 64), dtype=np.float32)
    for dd in range(32):
        r0[dd + 32, dd] = -1.0     # out[d<32] = -q[d+32]
        r0[dd, dd + 32] = 1.0      # out[d>=32] = q[d-32]
    rotm = np.zeros((128, 128), dtype=np.float32)
    rotm[:64, :64] = r0
    rotm[64:, 64:] = r0

    tt_, qq_ = np.meshgrid(np.arange(128), np.arange(128), indexing="ij")
    trim = (qq_ >= tt_).astype(np.float32)

    in_maps = []
    for c in range(N_CORES):
        b, g = c // 4, c % 4
        heads = [4 * g + hh for hh in range(HPC)]
        wqk = np.concatenate(
            [W_qkv[:, 64 * h:64 * h + 64] for h in heads]
            + [W_qkv[:, D + 64 * h:D + 64 * h + 64] for h in heads],
            axis=1,
        )
        wv = np.concatenate(
            [W_qkv[:, 2 * D + 64 * h:2 * D + 64 * h + 64] for h in heads], axis=1
        )
        in_maps.append(
            {
                "xt": _npbf(x[b].T),
                "wqk": _npbf(wqk),
                "wv": _npbf(wv),
                "wo": _npbf(W_o),
                "cosp": _npbf(cosp),
                "sinp": _npbf(sinp),
                "rotm": _npbf(rotm),
                "trim": _npbf(trim),
            }
        )
    return in_maps


TAIL_COLS = 1024


def assemble_output(results):
    full = np.empty((B, S, D), dtype=np.float32)
    for k in range(N_CORES):
        o = np.asarray(results[k]["out"], dtype=np.float32)  # [B, 256, D]
        for b in range(B):
            if TAIL_COLS == 512:
                full[b, 192 * k:192 * (k + 1)] = o[b, 0:192]
                full[b, 1536 + 64 * k:1536 + 64 * (k + 1)] = o[b, 192:256]
            else:
                full[b, 128 * k:128 * (k + 1)] = o[b, 0:128]
                full[b, 1024 + 128 * k:1024 + 128 * (k + 1)] = o[b, 128:256]
    return full


_NC_CACHE = {}


def kernel(x, W_qkv, W_o):
    key = "nc"
    if key not in _NC_CACHE:
        _NC_CACHE[key] = build_nc(tail_cols=TAIL_COLS)
    nc = _NC_CACHE[key]
    in_maps = prepare_in_maps(x, W_qkv, W_o)
    res = run_bass_kernel_spmd(nc, in_maps, core_ids=list(range(N_CORES)))
    return assemble_output(res.results)
